# revision 13
# baseline (speedup 1.0000x reference)
"""Causal GQA self-attention (b=4, s=2048, dim=2048, 16 q-heads / 4 kv-heads,
hd=128, RoPE) on 8 TRN2 NeuronCores.

Sharding: tensor-parallel x2 on heads x data-parallel x4 on batch.
Core c <-> (batch c//2, head-half c%2). Per core:
  - K (2 kv heads) / Q (8 heads) projections in [hd, tok] layout with RoPE
    applied via partition-swap DMAs + DVE combines, software-pipelined,
  - V projection in [tok, hd] layout; tiles 4..15 woven as PE filler into
    the qt=0 attention (which is otherwise exp-latency-bound),
  - attention in S^T = K^T.T@Q layout (k on partitions, q on free):
    exp on ScalarE batched per k-tile pair, causal mask restricted to the
    single 128-col triangle block per diagonal tile, row sums via a DVE
    pairwise tree + one GpSimd partition_all_reduce (no PE involvement),
    PV accumulated in PSUM with diagonal-restricted column ranges; the
    normalized attention output is written back into the dead q-chunk
    slot of q_sb (saves 4MB SBUF),
  - output projection matmuls woven between attention score pairs as PE
    filler granules (keeps the PE fed while ScalarE works on exp),
  - pairwise ReduceScatter(add) per 128-token tile, writing directly into
    the bf16 output tensor (host upcasts to f32), so the collective tail
    after the last matmul is a single 0.5MB-in slice.
"""

import os
from collections import deque

import numpy as np
import ml_dtypes

import concourse.bass as bass
import concourse.bacc as bacc
import concourse.mybir as mybir
import concourse.tile as tile
import concourse.bass_isa as bass_isa
from concourse.bass_utils import run_bass_kernel_spmd

BF16 = ml_dtypes.bfloat16
F32 = mybir.dt.float32
BF = mybir.dt.bfloat16

# Problem constants
B, S, DIM = 4, 2048, 2048
NH, NKV, HD = 16, 4, 128
ROPE_BASE = 10000.0
N_CORES = 8

# Per-core layout
NH_LOC = NH // 2          # 8 q heads per core
NKV_LOC = NKV // 2        # 2 kv heads per core
P = 128
NDT = DIM // P            # 16 contraction tiles
TQ = 512                  # q-chunk (free dim of attention matmuls)
NQT = S // TQ             # 4 q-chunks
NTT = S // P              # 16 token tiles of 128
SCALE = 1.0 / float(np.sqrt(HD))
QHALF = S // 2            # rows of per-core output (4 chunks x 4 x 64)

_PROGRAM_CACHE = {}


def _build_program():
    if "nc" in _PROGRAM_CACHE:
        return _PROGRAM_CACHE["nc"]

    nc = bacc.Bacc("TRN2", target_bir_lowering=False, debug=False,
                   num_devices=N_CORES)

    xT_d = nc.dram_tensor("xT", [DIM, S], BF, kind="ExternalInput")
    wqT_d = nc.dram_tensor("wqT", [DIM, NH_LOC * HD], BF, kind="ExternalInput")
    wkT_d = nc.dram_tensor("wkT", [DIM, NKV_LOC * HD], BF, kind="ExternalInput")
    wvT_d = nc.dram_tensor("wvT", [DIM, NKV_LOC * HD], BF, kind="ExternalInput")
    woT_d = nc.dram_tensor("woT", [NH_LOC * HD, DIM], BF, kind="ExternalInput")
    cos_d = nc.dram_tensor("cos", [P, S], BF, kind="ExternalInput")
    ssin_d = nc.dram_tensor("ssin", [P, S], BF, kind="ExternalInput")
    mask_d = nc.dram_tensor("mask", [P, P], BF, kind="ExternalInput")
    ones_d = nc.dram_tensor("ones", [P, 1], BF, kind="ExternalInput")
    onesr_d = nc.dram_tensor("onesr", [1, P], BF, kind="ExternalInput")
    out_d = nc.dram_tensor("out", [QHALF, DIM], BF, kind="ExternalOutput")

    with tile.TileContext(nc) as tc:
        with tc.tile_pool(name="const", bufs=1) as constp, \
             tc.tile_pool(name="acts", bufs=1) as actp, \
             tc.tile_pool(name="dram", bufs=1, space="DRAM") as dramp, \
             tc.tile_pool(name="wk", bufs=2) as pwk:
            # ---- constants ----
            cos_sb = constp.tile([P, S], BF, name="cos_sb")
            ssin_sb = constp.tile([P, S], BF, name="ssin_sb")
            mask_sb = constp.tile([P, P], BF, name="mask_sb")
            ones_sb = constp.tile([P, 1], BF, name="ones_sb")
            onesr_sb = constp.tile([1, P], BF, name="onesr_sb")
            nc.sync.dma_start(out=cos_sb[:, :], in_=cos_d[:, :])
            nc.sync.dma_start(out=ssin_sb[:, :], in_=ssin_d[:, :])
            nc.sync.dma_start(out=mask_sb[:, :], in_=mask_d[:, :])
            nc.sync.dma_start(out=ones_sb[:, :], in_=ones_d[:, :])
            nc.sync.dma_start(out=onesr_sb[:, :], in_=onesr_d[:, :])

            # ---- persistent activations ----
            q_sb = [actp.tile([P, S], BF, name=f"q{h}") for h in range(NH_LOC)]
            k_sb = [actp.tile([P, S], BF, name=f"k{g}") for g in range(NKV_LOC)]
            v_sb = [actp.tile([P, NKV_LOC * HD], BF, name=f"v{t}")
                    for t in range(NTT)]
            partial_dr = [dramp.tile([TQ, DIM], BF, name=f"part{qt}")
                          for qt in range(NQT)]
            red_dr = [dramp.tile([TQ // 2, DIM], BF, name=f"red{qt}")
                      for qt in range(NQT)]

            # ---- streamed inputs ----
            # tc.tile allocations are a LIFO stack: allocate in reverse
            # order of freeing (wv freed last ... wk freed first).
            # x as 64 [P, TQ] tiles (d, c) so whole chunks can be freed.
            wvt, wvt_free = [], []
            for i in range(NDT):
                t_, f_ = tc.tile([P, NKV_LOC * HD], BF, name=f"wv{i}")
                wvt.append(t_); wvt_free.append(f_)
            xt = [[None] * NQT for _ in range(NDT)]
            xt_free = [[None] * NQT for _ in range(NDT)]
            for c in (3, 2, 1, 0):
                for d in range(NDT):
                    t_, f_ = tc.tile([P, TQ], BF, name=f"x{d}_{c}")
                    xt[d][c] = t_
                    xt_free[d][c] = f_
            wqt, wqt_free = [], []
            wkt, wkt_free = [], []
            for i in range(NDT):
                t_, f_ = tc.tile([P, NH_LOC * HD], BF, name=f"wq{i}")
                wqt.append(t_); wqt_free.append(f_)
            for i in range(NDT):
                t_, f_ = tc.tile([P, NKV_LOC * HD], BF, name=f"wk{i}")
                wkt.append(t_); wkt_free.append(f_)
            # DMA priority order: wk, x c0, wv, x c1, wq, x c2, x c3
            for i in range(NDT):
                nc.sync.dma_start(out=wkt[i][:, :], in_=wkT_d[i * P:(i + 1) * P, :])
            for d in range(NDT):
                nc.sync.dma_start(out=xt[d][0][:, :], in_=xT_d[d * P:(d + 1) * P, 0:TQ])
            for i in range(NDT):
                nc.sync.dma_start(out=wvt[i][:, :], in_=wvT_d[i * P:(i + 1) * P, :])
            for d in range(NDT):
                nc.sync.dma_start(out=xt[d][1][:, :], in_=xT_d[d * P:(d + 1) * P, TQ:2 * TQ])
            for i in range(NDT):
                nc.sync.dma_start(out=wqt[i][:, :], in_=wqT_d[i * P:(i + 1) * P, :])
            for c in (2, 3):
                for d in range(NDT):
                    nc.sync.dma_start(out=xt[d][c][:, :],
                                      in_=xT_d[d * P:(d + 1) * P, c * TQ:(c + 1) * TQ])

            # attention-phase psum (outer): sT 2x2 banks + opv 2x1 = 6 banks
            with tc.tile_pool(name="p2ps", bufs=1, space="PSUM") as p2ps:

                # ============ phase 1: K, V(0..3), Q projections ============
                p1ps_cm = tc.tile_pool(name="p1ps", bufs=1, space="PSUM")
                p1ps = p1ps_cm.__enter__()

                def combine(ps, dst, c):
                    raw = pwk.tile([P, TQ], BF, name="raw", tag="raw", bufs=2)
                    nc.scalar.copy(raw[:, :], ps[:, :])
                    # rotate-half via partition-swap DMAs (sign lives in ssin)
                    rot = pwk.tile([P, TQ], BF, name="rot", tag="rot", bufs=2)
                    nc.sync.dma_start(out=rot[0:64, :], in_=raw[64:128, :])
                    nc.sync.dma_start(out=rot[64:128, :], in_=raw[0:64, :])
                    t1 = pwk.tile([P, TQ], BF, name="t1", tag="t1", bufs=2)
                    nc.vector.tensor_mul(t1[:, :], raw[:, :],
                                         cos_sb[:, c * TQ:(c + 1) * TQ])
                    t2 = pwk.tile([P, TQ], BF, name="t2", tag="t2", bufs=2)
                    nc.vector.tensor_mul(t2[:, :], rot[:, :],
                                         ssin_sb[:, c * TQ:(c + 1) * TQ])
                    nc.vector.tensor_add(dst[:, c * TQ:(c + 1) * TQ],
                                         t1[:, :], t2[:, :])

                pending = []

                def proj_job(w_tiles, col0, dst, c):
                    ps = p1ps.tile([P, TQ], F32, name="projps",
                                   tag="projps", bufs=2)
                    for d in range(NDT):
                        nc.tensor.matmul(
                            ps[:, :],
                            lhsT=w_tiles[d][:, col0:col0 + HD],
                            rhs=xt[d][c][:, :],
                            start=(d == 0), stop=(d == NDT - 1))
                    pending.append((ps, dst, c))
                    if len(pending) > 1:
                        combine(*pending.pop(0))

                def v_matmuls(t, d0, d1):
                    ps_t = v_ps[t]
                    c, col = t // 4, (t % 4) * P
                    for d in range(d0, d1):
                        nc.tensor.matmul(
                            ps_t[0][:, 0:NKV_LOC * HD],
                            lhsT=xt[d][c][:, col:col + P],
                            rhs=wvt[d][:, :],
                            start=(d == 0), stop=(d == NDT - 1))

                def v_finish(t):
                    nc.scalar.copy(v_sb[t][:, :], v_ps[t][0][:, 0:NKV_LOC * HD])
                    del v_ps[t]

                v_ps = {}

                def v_start(t):
                    v_ps[t] = [p1ps.tile([P, TQ], F32, name="vps",
                                         tag="projps", bufs=2)]

                # K jobs (c-major), then V tiles 0..3, then Q jobs
                for c in range(NQT):
                    for g in range(NKV_LOC):
                        proj_job(wkt, g * HD, k_sb[g], c)
                # free wk once both pending K combines drained
                while pending:
                    combine(*pending.pop(0))
                for f_ in reversed(wkt_free):
                    f_()
                for t in range(4):
                    v_start(t)
                    v_matmuls(t, 0, NDT)
                    v_finish(t)
                for h in range(NH_LOC):
                    for c in range(NQT):
                        proj_job(wqt, h * HD, q_sb[h], c)
                while pending:
                    combine(*pending.pop(0))
                # wq + x chunk 0 no longer needed (V 0..3 + all proj done)
                for f_ in reversed(wqt_free):
                    f_()
                for d in reversed(range(NDT)):
                    xt_free[d][0]()

                # attention work pool opens only now, into the freed space
                p2wk_cm = tc.tile_pool(name="p2wk", bufs=1)
                p2wk = p2wk_cm.__enter__()

                # V tiles 4..15 as weavable filler granules for qt=0 attention
                def v_granules():
                    items = []
                    for t in range(4, NTT):
                        items.append(lambda t=t: (v_start(t), v_matmuls(t, 0, 6)))
                        items.append(lambda t=t: v_matmuls(t, 6, 11))
                        items.append(lambda t=t: v_matmuls(t, 11, NDT))
                        items.append(lambda t=t: v_finish(t))
                    return items

                # ============ phase 2: attention ============
                # one-time zero of the two cycling sT psum tiles
                for _ in range(2):
                    z = p2ps.tile([P, 2 * TQ], F32, name="sT",
                                  tag="sT", bufs=2)
                    nc.vector.memset(z[:, :], 0.0)
                # dedicated pT tiles for the two diagonal pairs: never-exp-
                # written strips zeroed once, stay zero across reuse.
                for _ in range(2):
                    zd1 = p2wk.tile([P, 2 * TQ], BF, name="pTd1",
                                    tag="pTd1", bufs=2)
                    nc.vector.memset(zd1[:, TQ:TQ + P], 0.0)
                    zd2 = p2wk.tile([P, 2 * TQ], BF, name="pTd2",
                                    tag="pTd2", bufs=2)
                    nc.vector.memset(zd2[:, 0:2 * P], 0.0)
                    nc.vector.memset(zd2[:, TQ:TQ + 3 * P], 0.0)

                def attention(h, qt, pull):
                    """Scores/exp/mask/PV + row-sum + normalize for (h, qt).
                    Calls pull() between k-tile pairs to weave PE filler."""
                    g = h // (NH_LOC // NKV_LOC)
                    nk = (qt + 1) * (TQ // P)
                    npair = nk // 2
                    opv = p2ps.tile([P, TQ], F32, name="opv",
                                    tag="opv", bufs=1)
                    lvl = []
                    pend = []

                    def issue_pv(j, pT):
                        for i in range(2):
                            kt = 2 * j + i
                            dj = kt - 4 * qt
                            off = max(dj, 0) * P
                            nc.tensor.matmul(
                                opv[:, off:TQ],
                                lhsT=v_sb[kt][:, g * HD:(g + 1) * HD],
                                rhs=pT[:, i * TQ + off:(i + 1) * TQ],
                                start=(kt == 0), stop=(kt == nk - 1),
                                skip_group_check=True)

                    for j in range(npair):
                        sT = p2ps.tile([P, 2 * TQ], F32, name="sT",
                                       tag="sT", bufs=2)
                        if j == 2 * qt:
                            pT = p2wk.tile([P, 2 * TQ], BF, name="pTd1",
                                           tag="pTd1", bufs=2)
                        elif j == 2 * qt + 1:
                            pT = p2wk.tile([P, 2 * TQ], BF, name="pTd2",
                                           tag="pTd2", bufs=2)
                        else:
                            pT = p2wk.tile([P, 2 * TQ], BF, name="pT",
                                           tag="pT", bufs=4)
                        diag = j >= 2 * qt
                        for i in range(2):
                            kt = 2 * j + i
                            dj = kt - 4 * qt
                            off = max(dj, 0) * P
                            nc.tensor.matmul(
                                sT[:, i * TQ + off:(i + 1) * TQ],
                                lhsT=k_sb[g][:, kt * P:(kt + 1) * P],
                                rhs=q_sb[h][:, qt * TQ + off:(qt + 1) * TQ],
                                start=True, stop=True)
                        if not diag:
                            nc.scalar.activation(
                                pT[:, 0:2 * TQ], sT[:, 0:2 * TQ],
                                mybir.ActivationFunctionType.Exp, scale=SCALE)
                        else:
                            for i in range(2):
                                kt = 2 * j + i
                                dj = kt - 4 * qt
                                off = dj * P
                                nc.scalar.activation(
                                    pT[:, i * TQ + off:(i + 1) * TQ],
                                    sT[:, i * TQ + off:(i + 1) * TQ],
                                    mybir.ActivationFunctionType.Exp,
                                    scale=SCALE)
                                # causal triangle lives only in the first
                                # 128 cols past the diagonal offset
                                nc.vector.tensor_mul(
                                    pT[:, i * TQ + off:i * TQ + off + P],
                                    pT[:, i * TQ + off:i * TQ + off + P],
                                    mask_sb[:, :])
                        a = p2wk.tile([P, TQ], BF, name="sacc",
                                      tag="sacc", bufs=12)
                        nc.vector.tensor_add(a[:, :], pT[:, 0:TQ],
                                             pT[:, TQ:2 * TQ])
                        lvl.append(a)
                        pend.append((j, pT))
                        if len(pend) > 2:
                            issue_pv(*pend.pop(0))
                        pull()
                    while pend:
                        issue_pv(*pend.pop(0))

                    # deeper tree levels on DVE
                    while len(lvl) > 1:
                        nxt = []
                        for i in range(0, len(lvl) - 1, 2):
                            a = p2wk.tile([P, TQ], BF, name="sacc",
                                          tag="sacc", bufs=12)
                            nc.vector.tensor_add(a[:, :], lvl[i][:, :],
                                                 lvl[i + 1][:, :])
                            nxt.append(a)
                        if len(lvl) % 2:
                            nxt.append(lvl[-1])
                        lvl = nxt
                    acc = lvl[0]

                    def fin(pull=pull):
                        # row sums + partition broadcast on the PE (small
                        # matmuls), normalize on DVE into the dead q slot.
                        sums = p2ps.tile([P, TQ], F32, name="sums",
                                         tag="nrm", bufs=1)
                        nc.tensor.matmul(sums[0:1, :], lhsT=ones_sb[:, :],
                                         rhs=acc[:, :], start=True, stop=True)
                        rec = p2wk.tile([1, TQ], F32, name="rec",
                                        tag="rec", bufs=2)
                        nc.vector.reciprocal_approx_fast(rec[:, :],
                                                         sums[0:1, :])
                        rec16 = p2wk.tile([1, TQ], BF, name="rec16",
                                          tag="rec16", bufs=2)
                        nc.vector.tensor_scalar_mul(rec16[:, :], rec[:, :],
                                                    1.0)
                        pull()
                        recp = p2ps.tile([P, TQ], F32, name="recp",
                                         tag="nrm", bufs=1)
                        nc.tensor.matmul(recp[:, :], lhsT=onesr_sb[:, :],
                                         rhs=rec16[:, :], start=True,
                                         stop=True)
                        recb = p2wk.tile([P, TQ], BF, name="recb",
                                         tag="recb", bufs=2)
                        nc.scalar.copy(recb[:, :], recp[:, :])
                        nc.vector.tensor_mul(
                            q_sb[h][:, qt * TQ:(qt + 1) * TQ],
                            opv[:, :], recb[:, :])
                    return fin

                def rs_ts(qt, ts):
                    nc.gpsimd.collective_compute(
                        "ReduceScatter",
                        mybir.AluOpType.add,
                        replica_groups=[[2 * i, 2 * i + 1] for i in range(4)],
                        ins=[partial_dr[qt][ts * P:(ts + 1) * P, :].opt()],
                        outs=[red_dr[qt][ts * 64:(ts + 1) * 64, :].opt()],
                    )
                    nc.sync.dma_start(
                        out=out_d[qt * 256 + ts * 64:
                                  qt * 256 + ts * 64 + 64, :],
                        in_=red_dr[qt][ts * 64:(ts + 1) * 64, :])

                # ---- qt = 0 with V filler weave ----
                vq = deque(v_granules())

                def pull0():
                    if vq:
                        vq.popleft()()

                for h in range(NH_LOC):
                    fin = attention(h, 0, pull0)
                    pull0()
                    fin()
                    # drain V work to keep PE fed between heads
                    for _ in range(4):
                        pull0()
                while vq:
                    vq.popleft()()

                # close phase-1 psum; load wo (fits alongside the
                # remaining x chunks, which stay allocated to the end
                # per LIFO stack order).
                p1ps_cm.__exit__(None, None, None)

                wo = []
                wo_free = []
                for f in range(NH_LOC):
                    t_, f_ = tc.tile([P, DIM], BF, name=f"wo{f}")
                    wo.append(t_); wo_free.append(f_)
                    nc.sync.dma_start(out=t_[:, :],
                                      in_=woT_d[f * P:(f + 1) * P, :])

                with tc.tile_pool(name="p3ps", bufs=1, space="PSUM") as p3ps:

                    def oproj_items(qt):
                        """Filler granules computing the output projection of
                        chunk qt + per-ts ReduceScatter slices."""
                        items = []
                        for idx in range(16):
                            ts, cc = idx // 4, idx % 4
                            holder = {}

                            def mm(f0, f1, ts=ts, cc=cc, holder=holder,
                                   qt=qt):
                                if f0 == 0:
                                    holder["ps"] = p3ps.tile(
                                        [P, TQ], F32, name="ops",
                                        tag="ops", bufs=2)
                                ps = holder["ps"]
                                for f in range(f0, f1):
                                    nc.tensor.matmul(
                                        ps[:, :],
                                        lhsT=q_sb[f][:, qt * TQ + ts * P:
                                                     qt * TQ + (ts + 1) * P],
                                        rhs=wo[f][:, cc * TQ:(cc + 1) * TQ],
                                        start=(f == 0),
                                        stop=(f == NH_LOC - 1))

                            def fin(ts=ts, cc=cc, holder=holder, qt=qt):
                                posb = p2wk.tile([P, TQ], BF, name="posb",
                                                 tag="posb", bufs=4)
                                nc.vector.tensor_scalar_mul(
                                    posb[:, :], holder["ps"][:, :], 1.0)
                                nc.sync.dma_start(
                                    out=partial_dr[qt][ts * P:(ts + 1) * P,
                                                       cc * TQ:(cc + 1) * TQ],
                                    in_=posb[:, :])
                                if cc == 3:
                                    rs_ts(qt, ts)

                            items.append(lambda mm=mm: mm(0, 2))
                            items.append(lambda mm=mm: mm(2, 4))
                            items.append(lambda mm=mm: mm(4, 6))
                            items.append(lambda mm=mm: mm(6, 8))
                            items.append(fin)
                        return items

                    # ---- qt = 1..3 with oproj(qt-1) weave ----
                    for qt in range(1, NQT):
                        oq = deque(oproj_items(qt - 1))
                        quota = (len(oq) + NH_LOC - 1) // NH_LOC

                        def pull(oq=oq):
                            if oq:
                                oq.popleft()()

                        total = len(oq)
                        for h in range(NH_LOC):
                            fin = attention(h, qt, pull)
                            pull()
                            fin()
                            # head-end drain toward per-head quota
                            target = total - (h + 1) * quota
                            while len(oq) > max(target, 0):
                                oq.popleft()()
                        while oq:
                            oq.popleft()()

                    # ---- tail: oproj + RS of the last chunk ----
                    for it in oproj_items(NQT - 1):
                        it()

                for f_ in reversed(wo_free):
                    f_()
                p2wk_cm.__exit__(None, None, None)
                for c in (1, 2, 3):
                    for d in reversed(range(NDT)):
                        xt_free[d][c]()
                for f_ in reversed(wvt_free):
                    f_()

    nc.compile()
    _PROGRAM_CACHE["nc"] = nc
    return nc


def _host_tables():
    inv_freq = 1.0 / (ROPE_BASE ** (np.arange(0, HD, 2, dtype=np.float64) / HD))
    pos = np.arange(S, dtype=np.float64)
    ang = pos[None, :] * inv_freq[:, None]          # [64, S]
    cos = np.concatenate([np.cos(ang), np.cos(ang)], axis=0)   # [128, S]
    sin = np.sin(ang)
    ssin = np.concatenate([-sin, sin], axis=0)                  # [128, S]

    kk = np.arange(P)[:, None]
    cc = np.arange(P)[None, :]
    mask = (kk <= cc).astype(np.float32)                        # [128, 128]
    ones = np.ones((P, 1), np.float32)
    onesr = np.ones((1, P), np.float32)
    return (cos.astype(BF16), ssin.astype(BF16), mask.astype(BF16),
            ones.astype(BF16), onesr.astype(BF16))


def kernel(x, Wq, Wkv, Wo):
    x = np.asarray(x, np.float32)
    Wq = np.asarray(Wq, np.float32)
    Wkv = np.asarray(Wkv, np.float32)
    Wo = np.asarray(Wo, np.float32)

    nc = _build_program()
    cos, ssin, mask, ones, onesr = _host_tables()
    wqT = np.ascontiguousarray(Wq.T).astype(BF16)       # [DIM, 2048]
    wkvT = np.ascontiguousarray(Wkv.T).astype(BF16)     # [DIM, 1024]
    woT = np.ascontiguousarray(Wo.T).astype(BF16)       # [DIM, DIM]

    in_maps = []
    for c in range(N_CORES):
        b, hh = c // 2, c % 2
        xT = np.ascontiguousarray(x[b].T).astype(BF16)  # [DIM, S]
        in_maps.append({
            "xT": xT,
            "wqT": np.ascontiguousarray(
                wqT[:, hh * NH_LOC * HD:(hh + 1) * NH_LOC * HD]),
            "wkT": np.ascontiguousarray(
                wkvT[:, hh * NKV_LOC * HD:(hh + 1) * NKV_LOC * HD]),
            "wvT": np.ascontiguousarray(
                wkvT[:, NKV * HD + hh * NKV_LOC * HD:
                     NKV * HD + (hh + 1) * NKV_LOC * HD]),
            "woT": np.ascontiguousarray(
                woT[hh * NH_LOC * HD:(hh + 1) * NH_LOC * HD, :]),
            "cos": cos, "ssin": ssin, "mask": mask,
            "ones": ones, "onesr": onesr,
        })

    trace_kwargs = {}
    if os.environ.get("KERNEL_TRACE") == "1":
        trace_kwargs = dict(trace=True,
                            trace_cores=list(range(N_CORES)),
                            stitch_traces=True)
    elif os.environ.get("KERNEL_TRACE") == "0cores":
        trace_kwargs = dict(trace=True, trace_cores=[0])
    res = run_bass_kernel_spmd(nc, in_maps, core_ids=list(range(N_CORES)),
                               **trace_kwargs)
    _PROGRAM_CACHE["last_results"] = res

    out = np.empty((B, S, DIM), np.float32)
    for c in range(N_CORES):
        b, hh = c // 2, c % 2
        slab = res.results[c]["out"].astype(np.float32)  # [1024, 2048]
        for qt in range(NQT):
            for ts in range(4):
                t0 = qt * TQ + ts * P + hh * 64
                r0 = qt * 256 + ts * 64
                out[b, t0:t0 + 64, :] = slab[r0:r0 + 64]
    return out


# revision 23
# speedup vs baseline: 1.1879x; 1.1879x over previous
"""Causal GQA self-attention (b=4, s=2048, dim=2048, 16 q-heads / 4 kv-heads,
hd=128, RoPE) on 8 TRN2 NeuronCores.

Sharding: tensor-parallel x2 on heads x data-parallel x4 on batch.
Core c <-> (batch c//2, head-half c%2). Per core:
  - K (2 kv heads) / Q (8 heads) projections in [hd, tok] layout with RoPE
    applied via partition-swap DMAs + DVE combines, software-pipelined,
  - V projection in [tok, hd] layout; tiles 4..15 woven as PE filler into
    the qt=0 attention (which is otherwise exp-latency-bound),
  - attention in S^T = K^T.T@Q layout (k on partitions, q on free):
    exp on ScalarE batched per k-tile pair, causal mask restricted to the
    single 128-col triangle block per diagonal tile, row sums via a DVE
    pairwise tree + one GpSimd partition_all_reduce (no PE involvement),
    PV accumulated in PSUM with diagonal-restricted column ranges; the
    normalized attention output is written back into the dead q-chunk
    slot of q_sb (saves 4MB SBUF),
  - output projection matmuls woven between attention score pairs as PE
    filler granules (keeps the PE fed while ScalarE works on exp),
  - pairwise ReduceScatter(add) per 128-token tile, writing directly into
    the bf16 output tensor (host upcasts to f32), so the collective tail
    after the last matmul is a single 0.5MB-in slice.
"""

import os
from collections import deque

import numpy as np
import ml_dtypes

import concourse.bacc as bacc
import concourse.mybir as mybir
import concourse.tile as tile
from concourse.bass_utils import run_bass_kernel_spmd

BF16 = ml_dtypes.bfloat16
F32 = mybir.dt.float32
BF = mybir.dt.bfloat16

# Problem constants
B, S, DIM = 4, 2048, 2048
NH, NKV, HD = 16, 4, 128
ROPE_BASE = 10000.0
N_CORES = 8

# Per-core layout
NH_LOC = NH // 2          # 8 q heads per core
NKV_LOC = NKV // 2        # 2 kv heads per core
P = 128
NDT = DIM // P            # 16 contraction tiles
TQ = 512                  # q-chunk (free dim of attention matmuls)
NQT = S // TQ             # 4 q-chunks
NTT = S // P              # 16 token tiles of 128
SCALE = 1.0 / float(np.sqrt(HD))
QHALF = S // 2            # rows of per-core output (4 chunks x 4 x 64)

_PROGRAM_CACHE = {}


def _build_program():
    if "nc" in _PROGRAM_CACHE:
        return _PROGRAM_CACHE["nc"]

    nc = bacc.Bacc("TRN2", target_bir_lowering=False, debug=False,
                   num_devices=N_CORES)

    xT_d = nc.dram_tensor("xT", [DIM, S], BF, kind="ExternalInput")
    wqT_d = nc.dram_tensor("wqT", [DIM, NH_LOC * HD], BF, kind="ExternalInput")
    wkT_d = nc.dram_tensor("wkT", [DIM, NKV_LOC * HD], BF, kind="ExternalInput")
    wvT_d = nc.dram_tensor("wvT", [DIM, NKV_LOC * HD], BF, kind="ExternalInput")
    woT_d = nc.dram_tensor("woT", [NH_LOC * HD, DIM], BF, kind="ExternalInput")
    cos_d = nc.dram_tensor("cos", [P, S], BF, kind="ExternalInput")
    ssin_d = nc.dram_tensor("ssin", [P, S], BF, kind="ExternalInput")
    mask_d = nc.dram_tensor("mask", [P, P], BF, kind="ExternalInput")
    ones_d = nc.dram_tensor("ones", [P, 1], BF, kind="ExternalInput")
    onesr_d = nc.dram_tensor("onesr", [1, P], BF, kind="ExternalInput")
    out_d = nc.dram_tensor("out", [QHALF, DIM], BF, kind="ExternalOutput")

    with tile.TileContext(nc) as tc:
        with tc.tile_pool(name="const", bufs=1) as constp, \
             tc.tile_pool(name="acts", bufs=1) as actp, \
             tc.tile_pool(name="dram", bufs=1, space="DRAM") as dramp:
            # ---- constants ----
            mask_sb = constp.tile([P, P], BF, name="mask_sb")
            ones_sb = constp.tile([P, 1], BF, name="ones_sb")
            onesr_sb = constp.tile([1, P], BF, name="onesr_sb")
            nc.sync.dma_start(out=mask_sb[:, :], in_=mask_d[:, :])
            nc.sync.dma_start(out=ones_sb[:, :], in_=ones_d[:, :])
            nc.sync.dma_start(out=onesr_sb[:, :], in_=onesr_d[:, :])

            # ---- persistent activations ----
            q_sb = [actp.tile([P, S], BF, name=f"q{h}") for h in range(NH_LOC)]
            k_sb = [actp.tile([P, S], BF, name=f"k{g}") for g in range(NKV_LOC)]
            v_sb = [actp.tile([P, NKV_LOC * HD], BF, name=f"v{t}")
                    for t in range(NTT)]
            partial_dr = [dramp.tile([TQ, DIM], BF, name=f"part{qt}")
                          for qt in range(NQT)]
            red_dr = [dramp.tile([TQ // 2, DIM], BF, name=f"red{qt}")
                      for qt in range(NQT)]

            # ---- streamed inputs ----
            # tc.tile allocations are a LIFO stack: allocate in reverse
            # order of freeing (wv freed last ... wk freed first).
            # Weights live in single wide tiles ([P, d*cols] layout) loaded
            # with batched 3D-AP DMAs: a handful of SP ring slots instead
            # of 48, so the rot/posb DMAs behind them are never starved.
            KVW = NKV_LOC * HD
            QW = NH_LOC * HD
            wv_big, wv_free = tc.tile([P, NDT * KVW], BF, name="wv")
            xt, xt_free = [], []
            for d in range(NDT):
                t_, f_ = tc.tile([P, S], BF, name=f"x{d}")
                xt.append(t_); xt_free.append(f_)
            wq_big, wq_free = tc.tile([P, NDT * QW], BF, name="wq")
            cos_sb, cos_free = tc.tile([P, S], BF, name="cos_sb")
            ssin_sb, ssin_free = tc.tile([P, S], BF, name="ssin_sb")
            pwk_cm = tc.tile_pool(name="wk", bufs=2)
            pwk = pwk_cm.__enter__()
            wk_big, wk_free = tc.tile([P, NDT * KVW], BF, name="wk")
            # wk in 4 chunks so the first K matmul starts after 256KB,
            # then x tiles; cos/ssin are not needed until the first
            # combine, so they queue after the first x tiles.
            d4 = NDT // 4
            for j in range(4):
                nc.sync.dma_start(
                    out=wk_big[:, j * d4 * KVW:(j + 1) * d4 * KVW].rearrange(
                        "p (d c) -> p d c", d=d4),
                    in_=wkT_d[j * d4 * P:(j + 1) * d4 * P, :].rearrange(
                        "(d p) c -> p d c", p=P))
                nc.sync.dma_start(out=xt[j][:, :],
                                  in_=xT_d[j * P:(j + 1) * P, :])
            for i in range(4, NDT):
                nc.sync.dma_start(out=xt[i][:, :], in_=xT_d[i * P:(i + 1) * P, :])
            nc.sync.dma_start(out=cos_sb[:, :], in_=cos_d[:, :])
            nc.sync.dma_start(out=ssin_sb[:, :], in_=ssin_d[:, :])
            nc.sync.dma_start(
                out=wv_big[:, :].rearrange("p (d c) -> p d c", d=NDT),
                in_=wvT_d[:, :].rearrange("(d p) c -> p d c", p=P))
            for j in range(4):
                nc.sync.dma_start(
                    out=wq_big[:, j * d4 * QW:(j + 1) * d4 * QW].rearrange(
                        "p (d c) -> p d c", d=d4),
                    in_=wqT_d[j * d4 * P:(j + 1) * d4 * P, :].rearrange(
                        "(d p) c -> p d c", p=P))

            # attention-phase psum (outer): sT 2x2 banks + opv 2x1 = 6 banks
            with tc.tile_pool(name="p2ps", bufs=1, space="PSUM") as p2ps:

                # ============ phase 1: K, V(0..3), Q projections ============
                p1ps_cm = tc.tile_pool(name="p1ps", bufs=1, space="PSUM")
                p1ps = p1ps_cm.__enter__()

                def combine(ps, dst, c):
                    raw = pwk.tile([P, TQ], BF, name="raw", tag="raw", bufs=2)
                    nc.scalar.copy(raw[:, :], ps[:, :])
                    # rotate-half via partition-swap DMAs (sign lives in ssin)
                    rot = pwk.tile([P, TQ], BF, name="rot", tag="rot", bufs=2)
                    nc.gpsimd.dma_start(out=rot[0:64, :], in_=raw[64:128, :])
                    nc.gpsimd.dma_start(out=rot[64:128, :], in_=raw[0:64, :])
                    t1 = pwk.tile([P, TQ], BF, name="t1", tag="t1", bufs=2)
                    nc.vector.tensor_mul(t1[:, :], raw[:, :],
                                         cos_sb[:, c * TQ:(c + 1) * TQ])
                    t2 = pwk.tile([P, TQ], BF, name="t2", tag="t2", bufs=2)
                    nc.vector.tensor_mul(t2[:, :], rot[:, :],
                                         ssin_sb[:, c * TQ:(c + 1) * TQ])
                    nc.vector.tensor_add(dst[:, c * TQ:(c + 1) * TQ],
                                         t1[:, :], t2[:, :])

                pending = []

                def proj_job(w_big, wstride, col0, dst, c):
                    ps = p1ps.tile([P, TQ], F32, name="projps",
                                   tag="projps", bufs=2)
                    for d in range(NDT):
                        nc.tensor.matmul(
                            ps[:, :],
                            lhsT=w_big[:, d * wstride + col0:
                                       d * wstride + col0 + HD],
                            rhs=xt[d][:, c * TQ:(c + 1) * TQ],
                            start=(d == 0), stop=(d == NDT - 1))
                    pending.append((ps, dst, c))
                    if len(pending) > 1:
                        combine(*pending.pop(0))

                def v_matmuls(t, d0, d1):
                    ps_t = v_ps[t]
                    c, col = t // 4, (t % 4) * P
                    for d in range(d0, d1):
                        nc.tensor.matmul(
                            ps_t[0][:, 0:NKV_LOC * HD],
                            lhsT=xt[d][:, c * TQ + col:c * TQ + col + P],
                            rhs=wv_big[:, d * KVW:(d + 1) * KVW],
                            start=(d == 0), stop=(d == NDT - 1))

                def v_finish(t):
                    nc.scalar.copy(v_sb[t][:, :], v_ps[t][0][:, 0:NKV_LOC * HD])
                    del v_ps[t]

                v_ps = {}

                def v_start(t):
                    v_ps[t] = [p1ps.tile([P, TQ], F32, name="vps",
                                         tag="projps", bufs=2)]

                # K jobs (c-major), then V tiles 0..3, then Q jobs
                for c in range(NQT):
                    for g in range(NKV_LOC):
                        proj_job(wk_big, KVW, g * HD, k_sb[g], c)
                # free wk once both pending K combines drained
                while pending:
                    combine(*pending.pop(0))
                wk_free()
                for t in range(4):
                    v_start(t)
                    v_matmuls(t, 0, NDT)
                    v_finish(t)
                for h in range(NH_LOC):
                    for c in range(NQT):
                        proj_job(wq_big, QW, h * HD, q_sb[h], c)
                while pending:
                    combine(*pending.pop(0))
                pwk_cm.__exit__(None, None, None)
                ssin_free()
                cos_free()
                wq_free()

                # attention work pool opens only now, into the freed space
                p2wk_cm = tc.tile_pool(name="p2wk", bufs=1)
                p2wk = p2wk_cm.__enter__()

                # V tiles 4..15 as weavable filler granules for qt=0 attention
                def v_granules():
                    items = []
                    for t in range(4, NTT):
                        items.append(lambda t=t: (v_start(t), v_matmuls(t, 0, 6)))
                        items.append(lambda t=t: v_matmuls(t, 6, 11))
                        items.append(lambda t=t: v_matmuls(t, 11, NDT))
                        items.append(lambda t=t: v_finish(t))
                    return items

                # ============ phase 2: attention ============
                # one-time zero of the two cycling sT psum tiles
                for _ in range(2):
                    z = p2ps.tile([P, 2 * TQ], F32, name="sT",
                                  tag="sT", bufs=2)
                    nc.vector.memset(z[:, :], 0.0)
                # dedicated pT tiles for the two diagonal pairs: never-exp-
                # written strips zeroed once, stay zero across reuse.
                for _ in range(2):
                    zd1 = p2wk.tile([P, 2 * TQ], BF, name="pTd1",
                                    tag="pTd1", bufs=2)
                    nc.vector.memset(zd1[:, TQ:TQ + P], 0.0)
                    zd2 = p2wk.tile([P, 2 * TQ], BF, name="pTd2",
                                    tag="pTd2", bufs=2)
                    nc.vector.memset(zd2[:, 0:2 * P], 0.0)
                    nc.vector.memset(zd2[:, TQ:TQ + 3 * P], 0.0)

                att_cur = {}

                def attention(h, qt, pull):
                    """Scores/exp/mask/PV + row-sum + normalize for (h, qt).
                    Calls pull() between k-tile pairs to weave PE filler."""
                    g = h // (NH_LOC // NKV_LOC)
                    nk = (qt + 1) * (TQ // P)
                    npair = nk // 2
                    opv = p2ps.tile([P, TQ], F32, name="opv",
                                    tag="opv", bufs=1)
                    lvl = []
                    pend = []

                    def issue_pv(j, pT):
                        for i in range(2):
                            kt = 2 * j + i
                            dj = kt - 4 * qt
                            off = max(dj, 0) * P
                            nc.tensor.matmul(
                                opv[:, off:TQ],
                                lhsT=v_sb[kt][:, g * HD:(g + 1) * HD],
                                rhs=pT[:, i * TQ + off:(i + 1) * TQ],
                                start=(kt == 0), stop=(kt == nk - 1),
                                skip_group_check=True)

                    for j in range(npair):
                        sT = p2ps.tile([P, 2 * TQ], F32, name="sT",
                                       tag="sT", bufs=2)
                        if j == 2 * qt:
                            pT = p2wk.tile([P, 2 * TQ], BF, name="pTd1",
                                           tag="pTd1", bufs=2)
                        elif j == 2 * qt + 1:
                            pT = p2wk.tile([P, 2 * TQ], BF, name="pTd2",
                                           tag="pTd2", bufs=2)
                        else:
                            pT = p2wk.tile([P, 2 * TQ], BF, name="pT",
                                           tag="pT", bufs=3)
                        diag = j >= 2 * qt
                        for i in range(2):
                            kt = 2 * j + i
                            dj = kt - 4 * qt
                            off = max(dj, 0) * P
                            nc.tensor.matmul(
                                sT[:, i * TQ + off:(i + 1) * TQ],
                                lhsT=k_sb[g][:, kt * P:(kt + 1) * P],
                                rhs=q_sb[h][:, qt * TQ + off:(qt + 1) * TQ],
                                start=True, stop=True)
                        if not diag:
                            nc.scalar.activation(
                                pT[:, 0:2 * TQ], sT[:, 0:2 * TQ],
                                mybir.ActivationFunctionType.Exp, scale=SCALE)
                        else:
                            for i in range(2):
                                kt = 2 * j + i
                                dj = kt - 4 * qt
                                off = dj * P
                                nc.scalar.activation(
                                    pT[:, i * TQ + off:(i + 1) * TQ],
                                    sT[:, i * TQ + off:(i + 1) * TQ],
                                    mybir.ActivationFunctionType.Exp,
                                    scale=SCALE)
                                # causal triangle lives only in the first
                                # 128 cols past the diagonal offset
                                nc.vector.tensor_mul(
                                    pT[:, i * TQ + off:i * TQ + off + P],
                                    pT[:, i * TQ + off:i * TQ + off + P],
                                    mask_sb[:, :])
                        a = p2wk.tile([P, TQ], BF, name="sacc",
                                      tag="sacc", bufs=10)
                        nc.vector.tensor_add(a[:, :], pT[:, 0:TQ],
                                             pT[:, TQ:2 * TQ])
                        lvl.append(a)
                        pend.append((j, pT))
                        if len(pend) > 2:
                            issue_pv(*pend.pop(0))
                        pull()
                    while pend:
                        issue_pv(*pend.pop(0))

                    # deeper tree levels on DVE
                    while len(lvl) > 1:
                        nxt = []
                        for i in range(0, len(lvl) - 1, 2):
                            a = p2wk.tile([P, TQ], BF, name="sacc",
                                          tag="sacc", bufs=10)
                            nc.vector.tensor_add(a[:, :], lvl[i][:, :],
                                                 lvl[i + 1][:, :])
                            nxt.append(a)
                        if len(lvl) % 2:
                            nxt.append(lvl[-1])
                        lvl = nxt
                    acc = lvl[0]

                    def fin(pull=pull):
                        # row sums + partition broadcast on the PE (small
                        # matmuls), normalize on DVE into the dead q slot.
                        sums = p2ps.tile([P, TQ], F32, name="sums",
                                         tag="nrm", bufs=1)
                        nc.tensor.matmul(sums[0:1, :], lhsT=ones_sb[:, :],
                                         rhs=acc[:, :], start=True, stop=True)
                        rec = p2wk.tile([1, TQ], F32, name="rec",
                                        tag="rec", bufs=2)
                        nc.vector.reciprocal_approx_fast(rec[:, :],
                                                         sums[0:1, :])
                        rec16 = p2wk.tile([1, TQ], BF, name="rec16",
                                          tag="rec16", bufs=2)
                        nc.vector.tensor_scalar_mul(rec16[:, :], rec[:, :],
                                                    1.0)
                        pull()
                        recp = p2ps.tile([P, TQ], F32, name="recp",
                                         tag="nrm", bufs=1)
                        nc.tensor.matmul(recp[:, :], lhsT=onesr_sb[:, :],
                                         rhs=rec16[:, :], start=True,
                                         stop=True)
                        recb = p2wk.tile([P, TQ], BF, name="recb",
                                         tag="recb", bufs=2)
                        nc.scalar.copy(recb[:, :], recp[:, :])
                        att = p2wk.tile([P, TQ], BF, name=f"att{h}",
                                        tag=f"att{h}", bufs=2)
                        att_cur[h] = att
                        nc.vector.tensor_mul(att[:, :], opv[:, :],
                                             recb[:, :])
                    return fin

                RG = [[2 * i, 2 * i + 1] for i in range(4)]

                def rs_qt(qt):
                    # one collective per chunk: the ~10us fixed cost of a
                    # ReduceScatter dwarfs its bandwidth term, so fewer,
                    # bigger ops win. The out DMA is issued on the gpsimd
                    # DMA ring so its wait on the collective cannot
                    # head-of-line-block the SP ring that carries the
                    # partial_dr writes.
                    nc.gpsimd.collective_compute(
                        "ReduceScatter", mybir.AluOpType.add,
                        replica_groups=RG,
                        ins=[partial_dr[qt][:, :].opt()],
                        outs=[red_dr[qt][:, :].opt()],
                    )
                    nc.gpsimd.dma_start(
                        out=out_d[qt * 256:(qt + 1) * 256, :],
                        in_=red_dr[qt][:, :])

                def rs_part(qt, r0_in, nrows_in, defer_dma=False):
                    # rows [r0_in, r0_in+nrows_in) of partial -> each rank
                    # gets nrows_in/2 rows at red[r0_in/2:...]
                    r0, nr = r0_in // 2, nrows_in // 2
                    nc.gpsimd.collective_compute(
                        "ReduceScatter", mybir.AluOpType.add,
                        replica_groups=RG,
                        ins=[partial_dr[qt][r0_in:r0_in + nrows_in, :].opt()],
                        outs=[red_dr[qt][r0:r0 + nr, :].opt()],
                    )
                    def dma():
                        nc.gpsimd.dma_start(
                            out=out_d[qt * 256 + r0:qt * 256 + r0 + nr, :],
                            in_=red_dr[qt][r0:r0 + nr, :])
                    if defer_dma:
                        return dma
                    dma()

                # ---- qt = 0 with V filler weave ----
                vq = deque(v_granules())

                def pull0():
                    if vq:
                        vq.popleft()()

                for h in range(NH_LOC):
                    fin = attention(h, 0, pull0)
                    pull0()
                    fin()
                    # drain V work to keep PE fed between heads
                    for _ in range(4):
                        pull0()
                while vq:
                    vq.popleft()()

                # close phase-1 psum; load wo (fits alongside the
                # remaining x chunks, which stay allocated to the end
                # per LIFO stack order).
                p1ps_cm.__exit__(None, None, None)

                wo_big, wo_free = tc.tile([P, NH_LOC * DIM], BF, name="wo")
                for j in range(2):
                    f4 = NH_LOC // 2
                    nc.sync.dma_start(
                        out=wo_big[:, j * f4 * DIM:(j + 1) * f4 * DIM]
                        .rearrange("p (f c) -> p f c", f=f4),
                        in_=woT_d[j * f4 * P:(j + 1) * f4 * P, :]
                        .rearrange("(f p) c -> p f c", p=P))

                with tc.tile_pool(name="p3ps", bufs=1, space="PSUM") as p3ps:

                    def oproj_items(qt):
                        """Filler granules computing the output projection of
                        chunk qt + ReduceScatter slices."""
                        items = []
                        att_snap = dict(att_cur)
                        shared = {}
                        for idx in range(16):
                            ts, cc = idx // 4, idx % 4
                            holder = {"att": att_snap, "sh": shared}

                            def mm(f0, f1, ts=ts, cc=cc, holder=holder,
                                   qt=qt):
                                if f0 == 0:
                                    holder["ps"] = p3ps.tile(
                                        [P, TQ], F32, name="ops",
                                        tag="ops", bufs=2)
                                ps = holder["ps"]
                                att_prev = holder["att"]
                                for f in range(f0, f1):
                                    nc.tensor.matmul(
                                        ps[:, :],
                                        lhsT=att_prev[f][:, ts * P:
                                                         (ts + 1) * P],
                                        rhs=wo_big[:, f * DIM + cc * TQ:
                                                   f * DIM + (cc + 1) * TQ],
                                        start=(f == 0),
                                        stop=(f == NH_LOC - 1))

                            def fin(ts=ts, cc=cc, holder=holder, qt=qt):
                                posb = p2wk.tile([P, TQ], BF, name="posb",
                                                 tag="posb", bufs=3)
                                nc.scalar.copy(posb[:, :],
                                               holder["ps"][:, :])
                                nc.sync.dma_start(
                                    out=partial_dr[qt][ts * P:(ts + 1) * P,
                                                       cc * TQ:(cc + 1) * TQ],
                                    in_=posb[:, :])
                                if cc == 3:
                                    # last chunk: two half RS, out-DMAs
                                    # deferred so RS(3b) enqueues on the
                                    # gpsimd ring right behind RS(3a)
                                    if qt == NQT - 1 and ts == 1:
                                        holder["sh"]["dma0"] = rs_part(
                                            qt, 0, 256, defer_dma=True)
                                    elif qt == NQT - 1 and ts == 3:
                                        dma1 = rs_part(qt, 256, 256,
                                                       defer_dma=True)
                                        holder["sh"]["dma0"]()
                                        dma1()
                                    elif qt < NQT - 1 and ts == 3:
                                        rs_qt(qt)

                            items.append(lambda mm=mm: mm(0, 2))
                            items.append(lambda mm=mm: mm(2, 4))
                            items.append(lambda mm=mm: mm(4, 6))
                            items.append(lambda mm=mm: mm(6, 8))
                            items.append(fin)
                        return items

                    # ---- qt = 1..3 with oproj(qt-1) weave ----
                    for qt in range(1, NQT):
                        oq = deque(oproj_items(qt - 1))
                        quota = (len(oq) + NH_LOC - 1) // NH_LOC

                        def pull(oq=oq):
                            if oq:
                                oq.popleft()()

                        total = len(oq)
                        for h in range(NH_LOC):
                            fin = attention(h, qt, pull)
                            pull()
                            fin()
                            # head-end drain toward per-head quota
                            target = total - (h + 1) * quota
                            while len(oq) > max(target, 0):
                                oq.popleft()()
                        while oq:
                            oq.popleft()()

                    # ---- tail: oproj + RS of the last chunk ----
                    for it in oproj_items(NQT - 1):
                        it()

                wo_free()
                p2wk_cm.__exit__(None, None, None)
                for f_ in reversed(xt_free):
                    f_()
                wv_free()

    nc.compile()
    _PROGRAM_CACHE["nc"] = nc
    return nc


def _host_tables():
    inv_freq = 1.0 / (ROPE_BASE ** (np.arange(0, HD, 2, dtype=np.float64) / HD))
    pos = np.arange(S, dtype=np.float64)
    ang = pos[None, :] * inv_freq[:, None]          # [64, S]
    cos = np.concatenate([np.cos(ang), np.cos(ang)], axis=0)   # [128, S]
    sin = np.sin(ang)
    ssin = np.concatenate([-sin, sin], axis=0)                  # [128, S]

    kk = np.arange(P)[:, None]
    cc = np.arange(P)[None, :]
    mask = (kk <= cc).astype(np.float32)                        # [128, 128]
    ones = np.ones((P, 1), np.float32)
    onesr = np.ones((1, P), np.float32)
    return (cos.astype(BF16), ssin.astype(BF16), mask.astype(BF16),
            ones.astype(BF16), onesr.astype(BF16))


def kernel(x, Wq, Wkv, Wo):
    x = np.asarray(x, np.float32)
    Wq = np.asarray(Wq, np.float32)
    Wkv = np.asarray(Wkv, np.float32)
    Wo = np.asarray(Wo, np.float32)

    nc = _build_program()
    cos, ssin, mask, ones, onesr = _host_tables()
    wqT = np.ascontiguousarray(Wq.T).astype(BF16)       # [DIM, 2048]
    wkvT = np.ascontiguousarray(Wkv.T).astype(BF16)     # [DIM, 1024]
    woT = np.ascontiguousarray(Wo.T).astype(BF16)       # [DIM, DIM]

    in_maps = []
    for c in range(N_CORES):
        b, hh = c // 2, c % 2
        xT = np.ascontiguousarray(x[b].T).astype(BF16)  # [DIM, S]
        in_maps.append({
            "xT": xT,
            "wqT": np.ascontiguousarray(
                wqT[:, hh * NH_LOC * HD:(hh + 1) * NH_LOC * HD]),
            "wkT": np.ascontiguousarray(
                wkvT[:, hh * NKV_LOC * HD:(hh + 1) * NKV_LOC * HD]),
            "wvT": np.ascontiguousarray(
                wkvT[:, NKV * HD + hh * NKV_LOC * HD:
                     NKV * HD + (hh + 1) * NKV_LOC * HD]),
            "woT": np.ascontiguousarray(
                woT[hh * NH_LOC * HD:(hh + 1) * NH_LOC * HD, :]),
            "cos": cos, "ssin": ssin, "mask": mask,
            "ones": ones, "onesr": onesr,
        })

    trace_kwargs = {}
    if os.environ.get("KERNEL_TRACE") == "1":
        trace_kwargs = dict(trace=True,
                            trace_cores=list(range(N_CORES)),
                            stitch_traces=True)
    elif os.environ.get("KERNEL_TRACE") == "0cores":
        trace_kwargs = dict(trace=True, trace_cores=[0])
    res = run_bass_kernel_spmd(nc, in_maps, core_ids=list(range(N_CORES)),
                               **trace_kwargs)
    _PROGRAM_CACHE["last_results"] = res

    out = np.empty((B, S, DIM), np.float32)
    for c in range(N_CORES):
        b, hh = c // 2, c % 2
        slab = res.results[c]["out"].astype(np.float32)  # [1024, 2048]
        for qt in range(NQT):
            if qt < NQT - 1:
                # one RS over 512 rows: rank gets 256 rows
                t0 = qt * TQ + hh * 256
                r0 = qt * 256
                out[b, t0:t0 + 256, :] = slab[r0:r0 + 256]
            else:
                # two RS halves over 256 rows each: rank gets 128 rows
                for half in range(2):
                    t0 = qt * TQ + half * 256 + hh * P
                    r0 = qt * 256 + half * P
                    out[b, t0:t0 + P, :] = slab[r0:r0 + P]
    return out
